# revision 13
# baseline (speedup 1.0000x reference)
"""GATNet (3-layer GAT + BN + global mean pool + MLP head) on 8 Trainium2 cores.

Sharding strategy:
  - Nodes sharded contiguously across 8 cores (6250 real + 22 pad rows each).
  - Edges bucketed on host by (dst core, 128-node dst window).
  - Per window: per-128-edge-tile indirect DMA gathers of [h | a_src] rows
    (1040B) from a replicated DRAM table; attention logits assembled per
    edge; one-hot selection matmuls (PE) accumulate numerator + softmax
    denominator into a PSUM window, written out contiguously (no scatter).
  - h table replicated each layer via chunked AllGathers that overlap the
    edge phase (chunk-major node-id layout keeps AG output blocks
    contiguous).
  - Segment-softmax max-subtraction is skipped: the num/den ratio is
    mathematically identical and logits are tiny for this model.
"""

import math
from contextlib import ExitStack
from dataclasses import dataclass

import numpy as np

WN = 128          # window size (nodes) == PE tile width
HC = 256          # hidden channels (H*C)
HCA = HC + 4      # h row + a_src
H = 4             # heads
C = 64            # per-head channels
INA = 5           # augmented input rows for layer 0 (4 features + ones)
NCLS = 2
BN_EPS = 1e-5
NEG_SLOPE = 0.2


@dataclass
class Cfg:
    NC: int = 8            # cores
    N: int = 50000         # real nodes
    G: int = 512           # graphs
    NLOC: int = 6250       # real nodes per core
    NPAD: int = 6272       # padded nodes per core (multiple of WN and CH)
    CH: int = 7            # AllGather chunks per layer
    PT: int = 640          # pooling table rows (>= max gbase + 128)
    GBUF: int = 6          # in-flight indirect gathers

    @property
    def NW(self):
        return self.NPAD // WN

    @property
    def CW(self):
        return self.NW // self.CH

    @property
    def CHR(self):
        return self.NPAD // self.CH

    @property
    def PID_TOT(self):
        return self.NC * self.NPAD


FULL = Cfg()


# ---------------------------------------------------------------- host side
def _flatten_params(params, prefix=""):
    out = {}
    for k, v in params.items():
        if isinstance(v, dict):
            out.update(_flatten_params(v, prefix + k + "."))
        elif isinstance(v, list):
            for i, vi in enumerate(v):
                out.update(_flatten_params(vi, f"{prefix}{k}.{i}."))
        else:
            out[prefix + k] = v
    return out


def _preprocess(cfg: Cfg, x, edge_index, batch, params):
    NC, NLOC, NPAD, NW, CHR = cfg.NC, cfg.NLOC, cfg.NPAD, cfg.NW, cfg.CHR
    E = edge_index.shape[1]

    src = np.asarray(edge_index[0], np.int64)
    dst = np.asarray(edge_index[1], np.int64)
    x = np.asarray(x, np.float32)
    batch = np.asarray(batch, np.int64)

    # chunk-major padded node id (h-table row): pid = k*(NC*CHR) + r*CHR + j
    def pid_of(n):
        r = n // NLOC
        l = n % NLOC
        return (l // CHR) * (NC * CHR) + r * CHR + (l % CHR)

    pid_s = pid_of(src)
    r_d = dst // NLOC
    l_d = dst % NLOC
    w_d = l_d // WN
    rel_d = l_d % WN

    key = r_d * NW + w_d
    nkey = NC * NW
    order = np.argsort(key, kind="stable")
    cnt = np.bincount(key, minlength=nkey)
    start = np.zeros(nkey, np.int64)
    np.cumsum(cnt[:-1], out=start[1:])
    posin = np.arange(E, dtype=np.int64) - start[key[order]]

    T = max(1, int(math.ceil(cnt.max() / WN)))

    rr, ww, pp = r_d[order], w_d[order], posin
    idxs = np.zeros((NC, NW, 128, T), np.int32)
    drc = np.full((NC, NW, 128, T), -1.0, np.float32)
    idxs[rr, ww, pp % 128, pp // 128] = pid_s[order].astype(np.int32)
    drc[rr, ww, pp % 128, pp // 128] = rel_d[order].astype(np.float32)

    # x, transposed + augmented with a ones row, per core
    xT = np.zeros((NC, INA, NPAD), np.float32)
    xr = x.reshape(NC, NLOC, -1)
    xT[:, : x.shape[1], :NLOC] = np.transpose(xr, (0, 2, 1))
    xT[:, INA - 1, :NLOC] = 1.0

    # pooling metadata
    bl2 = batch.reshape(NC, NLOC)
    gbase = bl2[:, 0]
    brel = (bl2 - gbase[:, None]).astype(np.float32)
    assert brel.max() < 128, "graph window overflow"
    l_all = np.arange(NLOC)
    gidrel = np.full((NC, 128, NW), -1.0, np.float32)
    gidrel[:, l_all % WN, l_all // WN] = brel

    gscat = np.zeros((NC, 128, 1), np.int32)
    i128 = np.arange(128)
    for r in range(NC):
        gscat[r, :, 0] = gbase[r] + i128
    assert int(gbase.max()) + 127 < cfg.PT

    # parameters
    p = _flatten_params(params)
    p = {k: np.asarray(v, np.float32) for k, v in p.items()}

    def gs_cb(pref):
        gs = p[pref + "gamma"] / np.sqrt(p[pref + "var"] + BN_EPS)
        cb = p[pref + "beta"] - p[pref + "mean"] * gs
        return gs.astype(np.float32), cb.astype(np.float32)

    def a_both(i):
        A = np.zeros((HC, 2 * H), np.float32)
        for h in range(H):
            A[h * C:(h + 1) * C, h] = p[f"layers.{i}.att_src"][h]
            A[h * C:(h + 1) * C, H + h] = p[f"layers.{i}.att_dst"][h]
        return A

    gs0, cb0 = gs_cb("in_bn.")
    W0 = p["layers.0.W"]
    W0aug = np.vstack([gs0[:, None] * W0, (cb0 @ W0)[None, :]])
    rhs0 = np.hstack([W0aug, W0aug @ a_both(0)]).astype(np.float32)

    rhs12 = []
    for i in (1, 2):
        W = p[f"layers.{i}.W"]
        rhs12.append(np.hstack([W, W @ a_both(i)]).astype(np.float32))

    drs = np.zeros((3, HC), np.float32)
    drb = np.zeros((3, HC), np.float32)
    for i in range(3):
        gs, cb = gs_cb(f"layers.{i}.bn_")
        drs[i] = gs
        drb[i] = p[f"layers.{i}.bias"] * gs + cb

    shared = {
        "rhs0": rhs0, "rhs1": rhs12[0], "rhs2": rhs12[1],
        "rhsF": p["final_W"], "drs": drs, "drb": drb,
        "bf": p["final_b"][None, :], "W1": p["cls_W1"],
        "b1": p["cls_b1"][None, :], "W2": p["cls_W2"],
        "b2": p["cls_b2"][None, :],
    }

    in_maps = []
    for r in range(NC):
        m = {"xT": np.ascontiguousarray(xT[r]),
             "gidrel": np.ascontiguousarray(gidrel[r]),
             "gscat": np.ascontiguousarray(gscat[r]),
             "idxs": np.ascontiguousarray(idxs[r]),
             "drc": np.ascontiguousarray(drc[r])}
        m.update(shared)
        in_maps.append(m)
    return in_maps, T


# ---------------------------------------------------------------- device side
def _split_multi_waits(nc):
    """This container's walrus encodes at most one sync-wait per
    instruction; move extra waits onto carrier NOPs inserted just before,
    on the same engine."""
    from concourse import mybir

    for bb in nc.main_func.blocks:
        insts = bb.instructions
        out = []
        for ins in insts:
            si = ins.sync_info
            if si is not None and len(si.on_wait) > 1:
                waits = list(si.on_wait)
                del si.on_wait[1:]
                for w in waits[1:]:
                    n = mybir.InstNoOp(
                        name=f"{ins.name}.w{len(out)}", ins=[], outs=[])
                    n.engine = ins.engine
                    n.sync_info = mybir.SyncInfo(on_wait=[w], on_update=[])
                    out.append(n)
            out.append(ins)
        if len(out) != len(insts):
            insts.clear()
            insts.extend(out)


def _patch_tile_drain():
    """The kernel-tail Drain carries one sem-wait per DMA lane; this
    container's walrus only encodes a single sync-wait on TPB_CTRL
    instructions. Split the waits across preceding NOPs."""
    from concourse import mybir, tile

    if getattr(tile.TileContext, "_drain_patched", False):
        return

    def _drain_and_barrier(self, tick_clock, wait_clock):
        nc = self.nc
        carrier = nc.sync.nop(nofuse=True)
        wait_clock.add_sem_waits(
            carrier.ins, tile.ScopedClock({None: tick_clock.global_clock}))
        si = carrier.ins.sync_info
        if si is not None and len(si.on_wait) > 1:
            waits = list(si.on_wait)
            del si.on_wait[1:]
            for w in waits[1:]:
                n2 = nc.sync.nop(nofuse=True)
                si2 = n2.ins.sync_info
                if si2 is None:
                    n2.ins.sync_info = mybir.SyncInfo(on_wait=[w],
                                                      on_update=[])
                else:
                    si2.on_wait.append(w)
        nc.sync.drain()
        nc.all_engine_barrier()
        popped = nc._tile_sem_poison_stack.pop()
        assert popped is self._sem_poison
        nc.clear_and_free_semaphores(list(self.sems.allocated().values()))
        nc.all_engine_barrier()

    tile.TileContext._drain_and_barrier = _drain_and_barrier
    tile.TileContext._drain_patched = True


def _build(cfg: Cfg, T: int, split_waits: bool = True):
    from concourse import bass, mybir, tile
    from concourse.masks import make_identity

    _patch_tile_drain()

    dt = mybir.dt
    f32, i32 = dt.float32, dt.int32
    Alu = mybir.AluOpType
    Act = mybir.ActivationFunctionType
    IOff = bass.IndirectOffsetOnAxis

    NC, NW, CW, CHR = cfg.NC, cfg.NW, cfg.CW, cfg.CHR
    NPAD, PID_TOT, PT = cfg.NPAD, cfg.PID_TOT, cfg.PT
    RG = [list(range(NC))]
    NHW = HC + 2 * H  # 264
    NGT = (cfg.G + 127) // 128

    nc = bass.Bass()

    def din(name, shape, dtype=f32):
        return nc.dram_tensor(name, shape, dtype, kind="ExternalInput")

    xT_d = din("xT", [INA, NPAD])
    gidrel_d = din("gidrel", [128, NW])
    gscat_d = din("gscat", [128, 1], i32)
    idxs_d = din("idxs", [NW, 128, T], i32)
    drc_d = din("drc", [NW, 128, T])
    rhs0_d = din("rhs0", [INA, NHW])
    rhs1_d = din("rhs1", [HC, NHW])
    rhs2_d = din("rhs2", [HC, NHW])
    rhsF_d = din("rhsF", [HC, C])
    drs_d = din("drs", [3, HC])
    drb_d = din("drb", [3, HC])
    bf_d = din("bf", [1, C])
    W1_d = din("W1", [C, C])
    b1_d = din("b1", [1, C])
    W2_d = din("W2", [C, NCLS])
    b2_d = din("b2", [1, NCLS])
    out_d = nc.dram_tensor("out", [cfg.G, NCLS], f32, kind="ExternalOutput")

    h_loc = nc.dram_tensor("h_loc", [NPAD, HCA], f32)
    h_ext = [nc.dram_tensor(f"h_ext{i}", [PID_TOT, HCA], f32)
             for i in range(2)]
    pool_g = nc.dram_tensor("pool_g", [PT, 128], f32)
    pool_red = nc.dram_tensor("pool_red", [PT, 128], f32)

    with tile.TileContext(nc) as tc, ExitStack() as ctx:
        cpool = ctx.enter_context(tc.tile_pool(name="consts", bufs=1))
        sp = ctx.enter_context(tc.tile_pool(name="sbuf", bufs=2))
        sp3 = ctx.enter_context(tc.tile_pool(name="sbuf3", bufs=3))
        gpl = ctx.enter_context(tc.tile_pool(name="gp", bufs=cfg.GBUF))
        pp = ctx.enter_context(tc.tile_pool(name="psum", bufs=2, space="PSUM"))
        pp1 = ctx.enter_context(
            tc.tile_pool(name="psum1", bufs=1, space="PSUM"))

        # ---- constants
        ident = cpool.tile([128, 128], f32, tag="ident")
        make_identity(nc, ident[:])
        iota_row_i = cpool.tile([128, 128], i32, tag="iri")
        nc.gpsimd.iota(iota_row_i[:], pattern=[[1, 128]], base=0,
                       channel_multiplier=0)
        iota_row = cpool.tile([128, 128], f32, tag="irf")
        nc.vector.tensor_copy(iota_row[:], iota_row_i[:])

        xT = cpool.tile([INA, NPAD], f32, tag="xT")
        nc.sync.dma_start(out=xT[:], in_=xT_d[:])
        rhs0 = cpool.tile([INA, NHW], f32, tag="rhs0")
        nc.sync.dma_start(out=rhs0[:], in_=rhs0_d[:])
        rhs12 = []
        for i, d in ((0, rhs1_d), (1, rhs2_d)):
            t = cpool.tile([128, 2, NHW], f32, tag=f"rhs{i + 1}",
                           name=f"rhs{i + 1}t")
            nc.sync.dma_start(out=t[:, 0, :], in_=d[0:128, :])
            nc.sync.dma_start(out=t[:, 1, :], in_=d[128:256, :])
            rhs12.append(t)
        rhsF = cpool.tile([128, 2, C], f32, tag="rhsF")
        nc.sync.dma_start(out=rhsF[:, 0, :], in_=rhsF_d[0:128, :])
        nc.sync.dma_start(out=rhsF[:, 1, :], in_=rhsF_d[128:256, :])
        drs = cpool.tile([128, 3, HC], f32, tag="drs")
        drb = cpool.tile([128, 3, HC], f32, tag="drb")
        for i in range(3):
            nc.sync.dma_start(out=drs[:, i, :],
                              in_=drs_d[i:i + 1, :].to_broadcast([128, HC]))
            nc.sync.dma_start(out=drb[:, i, :],
                              in_=drb_d[i:i + 1, :].to_broadcast([128, HC]))
        bfr = cpool.tile([128, C], f32, tag="bfr")
        nc.sync.dma_start(out=bfr[:], in_=bf_d[:].to_broadcast([128, C]))
        W1 = cpool.tile([C, C], f32, tag="W1")
        nc.sync.dma_start(out=W1[:], in_=W1_d[:])
        b1r = cpool.tile([128, C], f32, tag="b1r")
        nc.sync.dma_start(out=b1r[:], in_=b1_d[:].to_broadcast([128, C]))
        W2 = cpool.tile([C, NCLS], f32, tag="W2")
        nc.sync.dma_start(out=W2[:], in_=W2_d[:])
        b2r = cpool.tile([128, NCLS], f32, tag="b2r")
        nc.sync.dma_start(out=b2r[:], in_=b2_d[:].to_broadcast([128, NCLS]))
        gidrel = cpool.tile([128, NW], f32, tag="gidrel")
        nc.sync.dma_start(out=gidrel[:], in_=gidrel_d[:])
        gscat = cpool.tile([128, 1], i32, tag="gscat")
        nc.sync.dma_start(out=gscat[:], in_=gscat_d[:])

        adst = [cpool.tile([128, NW * 4], f32, tag=f"adst{i}",
                           name=f"adst{i}") for i in range(2)]
        pool_ps = pp1.tile([128, 65], f32, tag="pool")

        def drain_common(w, ha_ps, tgt):
            ha_t = sp.tile([128, HCA], f32, tag="ha_t")
            nc.vector.tensor_copy(ha_t[:], ha_ps[:, 0:HCA])
            nc.sync.dma_start(out=h_loc[w * WN:(w + 1) * WN, :], in_=ha_t[:])
            nc.vector.tensor_copy(adst[tgt][:, w * 4:(w + 1) * 4],
                                  ha_ps[:, HC + 4:HC + 8])

        def chunk_ags(w, tgt):
            if (w + 1) % CW == 0:
                k = w // CW
                nc.gpsimd.collective_compute(
                    "AllGather", Alu.bypass, replica_groups=RG,
                    ins=[h_loc[k * CHR:(k + 1) * CHR, :]],
                    outs=[h_ext[tgt][k * NC * CHR:(k + 1) * NC * CHR, :]])

        # -------- layer 0 prologue
        for w in range(NW):
            ha_ps = pp.tile([128, NHW], f32, tag="ha")
            nc.tensor.matmul(ha_ps[:], xT[:, w * WN:(w + 1) * WN], rhs0[:],
                             start=True, stop=True)
            drain_common(w, ha_ps, 0)
            chunk_ags(w, 0)

        # -------- conv layers
        for l in range(3):
            cur, nxt = l % 2, (l + 1) % 2
            hf = h_ext[cur]
            for w in range(NW):
                out_ps = pp.tile([128, HCA], f32, tag="outwin")
                ih = sp.tile([128, T], i32, tag="idxh")
                nc.sync.dma_start(out=ih[:], in_=idxs_d[w])
                dc = sp.tile([128, T], f32, tag="drc")
                nc.sync.dma_start(out=dc[:], in_=drc_d[w])
                for t in range(T):
                    g = gpl.tile([128, HCA], f32, tag="g")
                    nc.gpsimd.indirect_dma_start(
                        out=g[:], out_offset=None, in_=hf[:],
                        in_offset=IOff(ap=ih[:, t:t + 1], axis=0))
                    s2 = sp3.tile([128, 128], f32, tag="s2")
                    nc.vector.tensor_scalar(
                        s2[:], iota_row[:], dc[:, t:t + 1], None,
                        op0=Alu.is_equal)
                    s2t_ps = pp.tile([128, 128], f32, tag="ps128")
                    nc.tensor.transpose(s2t_ps[:], s2[:], ident[:])
                    s2t = sp3.tile([128, 128], f32, tag="s2t")
                    nc.vector.tensor_copy(s2t[:], s2t_ps[:])
                    adp = pp1.tile([128, 4], f32, tag="adp")
                    nc.tensor.matmul(adp[:], s2t[:],
                                     adst[cur][:, w * 4:(w + 1) * 4],
                                     start=True, stop=True)
                    z = sp3.tile([128, 4], f32, tag="z")
                    nc.vector.tensor_tensor(z[:], g[:, HC:HCA], adp[:],
                                            op=Alu.add)
                    z2 = sp3.tile([128, 4], f32, tag="z2")
                    nc.gpsimd.tensor_scalar_mul(z2[:], z[:], NEG_SLOPE)
                    z3 = sp3.tile([128, 4], f32, tag="z3")
                    nc.vector.tensor_tensor(z3[:], z[:], z2[:], op=Alu.max)
                    ex = sp3.tile([128, 4], f32, tag="ex")
                    nc.scalar.activation(ex[:], z3[:], Act.Exp)
                    msg = sp3.tile([128, HCA], f32, tag="msg")
                    nc.vector.tensor_tensor(
                        msg[:, 0:HC].rearrange("p (a b) -> p a b", a=H),
                        g[:, 0:HC].rearrange("p (a b) -> p a b", a=H),
                        ex[:].to_broadcast([128, H, C]), op=Alu.mult)
                    nc.vector.tensor_copy(msg[:, HC:HCA], ex[:])
                    nc.tensor.matmul(out_ps[:], s2[:], msg[:],
                                     start=(t == 0), stop=(t == T - 1))
                # ---- drain window w
                den = sp3.tile([128, 4], f32, tag="den")
                nc.vector.tensor_scalar(den[:], out_ps[:, HC:HCA], 1e-16,
                                        None, op0=Alu.add)
                rec = sp3.tile([128, 4], f32, tag="rec")
                nc.vector.reciprocal(rec[:], den[:])
                xw = sp.tile([128, HC], f32, tag="xw")
                nc.vector.tensor_tensor(
                    xw[:].rearrange("p (a b) -> p a b", a=H),
                    out_ps[:, 0:HC].rearrange("p (a b) -> p a b", a=H),
                    rec[:].to_broadcast([128, H, C]), op=Alu.mult)
                xs = sp.tile([128, HC], f32, tag="xs")
                nc.vector.tensor_tensor(xs[:], xw[:], drs[:, l, :],
                                        op=Alu.mult)
                xb = sp.tile([128, HC], f32, tag="xb")
                nc.vector.tensor_tensor(xb[:], xs[:], drb[:, l, :],
                                        op=Alu.add)
                x_t = sp.tile([128, HC], f32, tag="x_t")
                nc.scalar.activation(x_t[:], xb[:], Act.Relu)
                xTa = []
                for cki in range(2):
                    tp = pp.tile([128, 128], f32, tag="ps128")
                    nc.tensor.transpose(
                        tp[:], x_t[:, cki * 128:(cki + 1) * 128], ident[:])
                    xt_s = sp.tile([128, 128], f32, tag=f"xT{cki}",
                                   name=f"xTs{cki}")
                    nc.vector.tensor_copy(xt_s[:], tp[:])
                    xTa.append(xt_s)
                if l < 2:
                    ha_ps = pp.tile([128, NHW], f32, tag="ha")
                    for cki in range(2):
                        nc.tensor.matmul(ha_ps[:], xTa[cki][:],
                                         rhs12[l][:, cki, :],
                                         start=(cki == 0), stop=(cki == 1))
                    drain_common(w, ha_ps, nxt)
                    chunk_ags(w, nxt)
                else:
                    y_ps = pp.tile([128, C], f32, tag="ha")
                    for cki in range(2):
                        nc.tensor.matmul(y_ps[:], xTa[cki][:],
                                         rhsF[:, cki, :],
                                         start=(cki == 0), stop=(cki == 1))
                    yb = sp.tile([128, C], f32, tag="yb")
                    nc.vector.tensor_tensor(yb[:], y_ps[:], bfr[:],
                                            op=Alu.add)
                    ya = sp.tile([128, C + 1], f32, tag="ya")
                    nc.scalar.activation(ya[:, 0:C], yb[:], Act.Relu)
                    nc.gpsimd.memset(ya[:, C:C + 1], 1.0)
                    s2g = sp.tile([128, 128], f32, tag="s2g")
                    nc.vector.tensor_scalar(s2g[:], iota_row[:],
                                            gidrel[:, w:w + 1], None,
                                            op0=Alu.is_equal)
                    nc.tensor.matmul(pool_ps[:], s2g[:], ya[:],
                                     start=(w == 0), stop=(w == NW - 1),
                                     skip_group_check=True)

        # -------- pooling + classifier
        zt = sp.tile([128, 128], f32, tag="zt")
        nc.gpsimd.memset(zt[:], 0.0)
        for i in range(PT // 128):
            nc.sync.dma_start(out=pool_g[i * 128:(i + 1) * 128, :], in_=zt[:])
        psb = sp.tile([128, 128], f32, tag="psb")
        nc.gpsimd.memset(psb[:], 0.0)
        nc.vector.tensor_copy(psb[:, 0:65], pool_ps[:])
        nc.gpsimd.indirect_dma_start(
            out=pool_g[:], out_offset=IOff(ap=gscat[:], axis=0),
            in_=psb[:], in_offset=None, compute_op=Alu.add)
        nc.gpsimd.collective_compute(
            "AllReduce", Alu.add, replica_groups=RG,
            ins=[pool_g[:]], outs=[pool_red[:]])

        for gt in range(NGT):
            rows = min(128, cfg.G - gt * 128)
            pg = sp.tile([128, 65], f32, tag="pg")
            nc.sync.dma_start(out=pg[:],
                              in_=pool_red[gt * 128:gt * 128 + 128, 0:65])
            cntm = sp.tile([128, 1], f32, tag="cntm")
            nc.vector.tensor_scalar_max(cntm[:], pg[:, 64:65], 1.0)
            crec = sp.tile([128, 1], f32, tag="crec")
            nc.vector.reciprocal(crec[:], cntm[:])
            g_t = sp.tile([128, C], f32, tag="g_t")
            nc.vector.tensor_scalar(g_t[:], pg[:, 0:C], crec[:, 0:1], None,
                                    op0=Alu.mult)
            tp = pp.tile([128, 128], f32, tag="ps128")
            nc.tensor.transpose(tp[0:C, :], g_t[:], ident[:])
            gT = sp.tile([C, 128], f32, tag="gT")
            nc.vector.tensor_copy(gT[:], tp[0:C, :])
            z1 = pp.tile([128, C], f32, tag="ha")
            nc.tensor.matmul(z1[:], gT[:], W1[:], start=True, stop=True)
            r1b = sp.tile([128, C], f32, tag="r1b")
            nc.vector.tensor_tensor(r1b[:], z1[:], b1r[:], op=Alu.add)
            r1 = sp.tile([128, C], f32, tag="r1")
            nc.scalar.activation(r1[:], r1b[:], Act.Relu)
            tp2 = pp.tile([128, 128], f32, tag="ps128")
            nc.tensor.transpose(tp2[0:C, :], r1[:], ident[:])
            r1T = sp.tile([C, 128], f32, tag="r1T")
            nc.vector.tensor_copy(r1T[:], tp2[0:C, :])
            z2p = pp.tile([128, NCLS], f32, tag="ha")
            nc.tensor.matmul(z2p[:], r1T[:], W2[:], start=True, stop=True)
            ob = sp.tile([128, NCLS], f32, tag="ob")
            nc.vector.tensor_tensor(ob[:], z2p[:], b2r[:], op=Alu.add)
            nc.sync.dma_start(out=out_d[gt * 128:gt * 128 + rows, :],
                              in_=ob[0:rows, :])

    if split_waits:
        _split_multi_waits(nc)
    return nc


# ---------------------------------------------------------------- entry point
def kernel(x, edge_index, batch, params):
    cfg = FULL
    in_maps, T = _preprocess(cfg, x, edge_index, batch, params)
    nc = _build(cfg, T)
    from concourse.bass_utils import run_bass_kernel_spmd
    res = run_bass_kernel_spmd(nc, in_maps, list(range(cfg.NC)))
    return np.asarray(res.results[0]["out"], np.float32)


# revision 16
# speedup vs baseline: 1.0077x; 1.0077x over previous
"""GATNet (3-layer GAT + BN + global mean pool + MLP head) on 8 Trainium2 cores.

Sharding strategy:
  - Nodes sharded contiguously across 8 cores (6250 real + 22 pad rows each).
  - Edges bucketed on host by (dst core, 128-node dst window).
  - Per window: per-128-edge-tile indirect DMA gathers of [h | a_src] rows
    (1040B) from a replicated DRAM table; attention logits assembled per
    edge; one-hot selection matmuls (PE) accumulate numerator + softmax
    denominator into a PSUM window, written out contiguously (no scatter).
  - h table replicated each layer via chunked AllGathers that overlap the
    edge phase (chunk-major node-id layout keeps AG output blocks
    contiguous).
  - Segment-softmax max-subtraction is skipped: the num/den ratio is
    mathematically identical and logits are tiny for this model.
"""

import math
from contextlib import ExitStack
from dataclasses import dataclass

import numpy as np

WN = 128          # window size (nodes) == PE tile width
HC = 256          # hidden channels (H*C)
HCA = HC + 4      # h row + a_src
H = 4             # heads
C = 64            # per-head channels
INA = 5           # augmented input rows for layer 0 (4 features + ones)
NCLS = 2
BN_EPS = 1e-5
NEG_SLOPE = 0.2


@dataclass
class Cfg:
    NC: int = 8            # cores
    N: int = 50000         # real nodes
    G: int = 512           # graphs
    NLOC: int = 6250       # real nodes per core
    NPAD: int = 6272       # padded nodes per core (multiple of WN and CH)
    CH: int = 7            # AllGather chunks per layer
    PT: int = 640          # pooling table rows (>= max gbase + 128)
    GBUF: int = 10          # in-flight indirect gathers

    @property
    def NW(self):
        return self.NPAD // WN

    @property
    def CW(self):
        return self.NW // self.CH

    @property
    def CHR(self):
        return self.NPAD // self.CH

    @property
    def PID_TOT(self):
        return self.NC * self.NPAD


FULL = Cfg()


# ---------------------------------------------------------------- host side
def _flatten_params(params, prefix=""):
    out = {}
    for k, v in params.items():
        if isinstance(v, dict):
            out.update(_flatten_params(v, prefix + k + "."))
        elif isinstance(v, list):
            for i, vi in enumerate(v):
                out.update(_flatten_params(vi, f"{prefix}{k}.{i}."))
        else:
            out[prefix + k] = v
    return out


def _preprocess(cfg: Cfg, x, edge_index, batch, params):
    NC, NLOC, NPAD, NW, CHR = cfg.NC, cfg.NLOC, cfg.NPAD, cfg.NW, cfg.CHR
    E = edge_index.shape[1]

    src = np.asarray(edge_index[0], np.int64)
    dst = np.asarray(edge_index[1], np.int64)
    x = np.asarray(x, np.float32)
    batch = np.asarray(batch, np.int64)

    # chunk-major padded node id (h-table row): pid = k*(NC*CHR) + r*CHR + j
    def pid_of(n):
        r = n // NLOC
        l = n % NLOC
        return (l // CHR) * (NC * CHR) + r * CHR + (l % CHR)

    pid_s = pid_of(src)
    r_d = dst // NLOC
    l_d = dst % NLOC
    w_d = l_d // WN
    rel_d = l_d % WN

    key = r_d * NW + w_d
    nkey = NC * NW
    order = np.argsort(key, kind="stable")
    cnt = np.bincount(key, minlength=nkey)
    start = np.zeros(nkey, np.int64)
    np.cumsum(cnt[:-1], out=start[1:])
    posin = np.arange(E, dtype=np.int64) - start[key[order]]

    T = max(1, int(math.ceil(cnt.max() / WN)))
    cw2 = cnt.reshape(NC, NW).max(axis=0)
    Tw = np.maximum(1, np.ceil(cw2 / WN).astype(np.int64))

    rr, ww, pp = r_d[order], w_d[order], posin
    idxs = np.zeros((NC, NW, 128, T), np.int32)
    drc = np.full((NC, NW, 128, T), -1.0, np.float32)
    idxs[rr, ww, pp % 128, pp // 128] = pid_s[order].astype(np.int32)
    drc[rr, ww, pp % 128, pp // 128] = rel_d[order].astype(np.float32)

    # x, transposed + augmented with a ones row, per core
    xT = np.zeros((NC, INA, NPAD), np.float32)
    xr = x.reshape(NC, NLOC, -1)
    xT[:, : x.shape[1], :NLOC] = np.transpose(xr, (0, 2, 1))
    xT[:, INA - 1, :NLOC] = 1.0

    # pooling metadata
    bl2 = batch.reshape(NC, NLOC)
    gbase = bl2[:, 0]
    brel = (bl2 - gbase[:, None]).astype(np.float32)
    assert brel.max() < 128, "graph window overflow"
    l_all = np.arange(NLOC)
    gidrel = np.full((NC, 128, NW), -1.0, np.float32)
    gidrel[:, l_all % WN, l_all // WN] = brel

    gscat = np.zeros((NC, 128, 1), np.int32)
    i128 = np.arange(128)
    for r in range(NC):
        gscat[r, :, 0] = gbase[r] + i128
    assert int(gbase.max()) + 127 < cfg.PT

    # parameters
    p = _flatten_params(params)
    p = {k: np.asarray(v, np.float32) for k, v in p.items()}

    def gs_cb(pref):
        gs = p[pref + "gamma"] / np.sqrt(p[pref + "var"] + BN_EPS)
        cb = p[pref + "beta"] - p[pref + "mean"] * gs
        return gs.astype(np.float32), cb.astype(np.float32)

    def a_both(i):
        A = np.zeros((HC, 2 * H), np.float32)
        for h in range(H):
            A[h * C:(h + 1) * C, h] = p[f"layers.{i}.att_src"][h]
            A[h * C:(h + 1) * C, H + h] = p[f"layers.{i}.att_dst"][h]
        return A

    gs0, cb0 = gs_cb("in_bn.")
    W0 = p["layers.0.W"]
    W0aug = np.vstack([gs0[:, None] * W0, (cb0 @ W0)[None, :]])
    rhs0 = np.hstack([W0aug, W0aug @ a_both(0)]).astype(np.float32)

    rhs12 = []
    for i in (1, 2):
        W = p[f"layers.{i}.W"]
        rhs12.append(np.hstack([W, W @ a_both(i)]).astype(np.float32))

    drs = np.zeros((3, HC), np.float32)
    drb = np.zeros((3, HC), np.float32)
    for i in range(3):
        gs, cb = gs_cb(f"layers.{i}.bn_")
        drs[i] = gs
        drb[i] = p[f"layers.{i}.bias"] * gs + cb

    shared = {
        "rhs0": rhs0, "rhs1": rhs12[0], "rhs2": rhs12[1],
        "rhsF": p["final_W"], "drs": drs, "drb": drb,
        "bf": p["final_b"][None, :], "W1": p["cls_W1"],
        "b1": p["cls_b1"][None, :], "W2": p["cls_W2"],
        "b2": p["cls_b2"][None, :],
    }

    in_maps = []
    for r in range(NC):
        m = {"xT": np.ascontiguousarray(xT[r]),
             "gidrel": np.ascontiguousarray(gidrel[r]),
             "gscat": np.ascontiguousarray(gscat[r]),
             "idxs": np.ascontiguousarray(idxs[r]),
             "drc": np.ascontiguousarray(drc[r])}
        m.update(shared)
        in_maps.append(m)
    return in_maps, T, [int(v) for v in Tw]


# ---------------------------------------------------------------- device side
def _split_multi_waits(nc):
    """This container's walrus encodes at most one sync-wait per
    instruction; move extra waits onto carrier NOPs inserted just before,
    on the same engine."""
    from concourse import mybir

    for bb in nc.main_func.blocks:
        insts = bb.instructions
        out = []
        for ins in insts:
            si = ins.sync_info
            if si is not None and len(si.on_wait) > 1:
                waits = list(si.on_wait)
                del si.on_wait[1:]
                for w in waits[1:]:
                    n = mybir.InstNoOp(
                        name=f"{ins.name}.w{len(out)}", ins=[], outs=[])
                    n.engine = ins.engine
                    n.sync_info = mybir.SyncInfo(on_wait=[w], on_update=[])
                    out.append(n)
            out.append(ins)
        if len(out) != len(insts):
            insts.clear()
            insts.extend(out)


def _patch_tile_drain():
    """The kernel-tail Drain carries one sem-wait per DMA lane; this
    container's walrus only encodes a single sync-wait on TPB_CTRL
    instructions. Split the waits across preceding NOPs."""
    from concourse import mybir, tile

    if getattr(tile.TileContext, "_drain_patched", False):
        return

    def _drain_and_barrier(self, tick_clock, wait_clock):
        nc = self.nc
        carrier = nc.sync.nop(nofuse=True)
        wait_clock.add_sem_waits(
            carrier.ins, tile.ScopedClock({None: tick_clock.global_clock}))
        si = carrier.ins.sync_info
        if si is not None and len(si.on_wait) > 1:
            waits = list(si.on_wait)
            del si.on_wait[1:]
            for w in waits[1:]:
                n2 = nc.sync.nop(nofuse=True)
                si2 = n2.ins.sync_info
                if si2 is None:
                    n2.ins.sync_info = mybir.SyncInfo(on_wait=[w],
                                                      on_update=[])
                else:
                    si2.on_wait.append(w)
        nc.sync.drain()
        nc.all_engine_barrier()
        popped = nc._tile_sem_poison_stack.pop()
        assert popped is self._sem_poison
        nc.clear_and_free_semaphores(list(self.sems.allocated().values()))
        nc.all_engine_barrier()

    tile.TileContext._drain_and_barrier = _drain_and_barrier
    tile.TileContext._drain_patched = True


def _build(cfg: Cfg, T: int, Tw=None, split_waits: bool = True, layers_mult: int = 1):
    from concourse import bass, mybir, tile
    from concourse.masks import make_identity

    _patch_tile_drain()

    dt = mybir.dt
    f32, i32 = dt.float32, dt.int32
    Alu = mybir.AluOpType
    Act = mybir.ActivationFunctionType
    IOff = bass.IndirectOffsetOnAxis

    if Tw is None:
        Tw = [T] * cfg.NW
    NC, NW, CW, CHR = cfg.NC, cfg.NW, cfg.CW, cfg.CHR
    NPAD, PID_TOT, PT = cfg.NPAD, cfg.PID_TOT, cfg.PT
    RG = [list(range(NC))]
    NHW = HC + 2 * H  # 264
    NGT = (cfg.G + 127) // 128

    nc = bass.Bass()

    def din(name, shape, dtype=f32):
        return nc.dram_tensor(name, shape, dtype, kind="ExternalInput")

    xT_d = din("xT", [INA, NPAD])
    gidrel_d = din("gidrel", [128, NW])
    gscat_d = din("gscat", [128, 1], i32)
    idxs_d = din("idxs", [NW, 128, T], i32)
    drc_d = din("drc", [NW, 128, T])
    rhs0_d = din("rhs0", [INA, NHW])
    rhs1_d = din("rhs1", [HC, NHW])
    rhs2_d = din("rhs2", [HC, NHW])
    rhsF_d = din("rhsF", [HC, C])
    drs_d = din("drs", [3, HC])
    drb_d = din("drb", [3, HC])
    bf_d = din("bf", [1, C])
    W1_d = din("W1", [C, C])
    b1_d = din("b1", [1, C])
    W2_d = din("W2", [C, NCLS])
    b2_d = din("b2", [1, NCLS])
    out_d = nc.dram_tensor("out", [cfg.G, NCLS], f32, kind="ExternalOutput")

    h_loc = nc.dram_tensor("h_loc", [NPAD, HCA], f32)
    h_ext = [nc.dram_tensor(f"h_ext{i}", [PID_TOT, HCA], f32)
             for i in range(2)]
    pool_g = nc.dram_tensor("pool_g", [PT, 128], f32)
    pool_red = nc.dram_tensor("pool_red", [PT, 128], f32)

    with tile.TileContext(nc) as tc, ExitStack() as ctx:
        cpool = ctx.enter_context(tc.tile_pool(name="consts", bufs=1))
        sp = ctx.enter_context(tc.tile_pool(name="sbuf", bufs=2))
        sp3 = ctx.enter_context(tc.tile_pool(name="sbuf3", bufs=4))
        gpl = ctx.enter_context(tc.tile_pool(name="gp", bufs=cfg.GBUF))
        pp = ctx.enter_context(tc.tile_pool(name="psum", bufs=2, space="PSUM"))
        pp1 = ctx.enter_context(
            tc.tile_pool(name="psum1", bufs=1, space="PSUM"))

        # ---- constants
        ident = cpool.tile([128, 128], f32, tag="ident")
        make_identity(nc, ident[:])
        iota_row_i = cpool.tile([128, 128], i32, tag="iri")
        nc.gpsimd.iota(iota_row_i[:], pattern=[[1, 128]], base=0,
                       channel_multiplier=0)
        iota_row = cpool.tile([128, 128], f32, tag="irf")
        nc.vector.tensor_copy(iota_row[:], iota_row_i[:])

        xT = cpool.tile([INA, NPAD], f32, tag="xT")
        nc.sync.dma_start(out=xT[:], in_=xT_d[:])
        rhs0 = cpool.tile([INA, NHW], f32, tag="rhs0")
        nc.sync.dma_start(out=rhs0[:], in_=rhs0_d[:])
        rhs12 = []
        for i, d in ((0, rhs1_d), (1, rhs2_d)):
            t = cpool.tile([128, 2, NHW], f32, tag=f"rhs{i + 1}",
                           name=f"rhs{i + 1}t")
            nc.sync.dma_start(out=t[:, 0, :], in_=d[0:128, :])
            nc.sync.dma_start(out=t[:, 1, :], in_=d[128:256, :])
            rhs12.append(t)
        rhsF = cpool.tile([128, 2, C], f32, tag="rhsF")
        nc.sync.dma_start(out=rhsF[:, 0, :], in_=rhsF_d[0:128, :])
        nc.sync.dma_start(out=rhsF[:, 1, :], in_=rhsF_d[128:256, :])
        drs = cpool.tile([128, 3, HC], f32, tag="drs")
        drb = cpool.tile([128, 3, HC], f32, tag="drb")
        for i in range(3):
            nc.sync.dma_start(out=drs[:, i, :],
                              in_=drs_d[i:i + 1, :].to_broadcast([128, HC]))
            nc.sync.dma_start(out=drb[:, i, :],
                              in_=drb_d[i:i + 1, :].to_broadcast([128, HC]))
        bfr = cpool.tile([128, C], f32, tag="bfr")
        nc.sync.dma_start(out=bfr[:], in_=bf_d[:].to_broadcast([128, C]))
        W1 = cpool.tile([C, C], f32, tag="W1")
        nc.sync.dma_start(out=W1[:], in_=W1_d[:])
        b1r = cpool.tile([128, C], f32, tag="b1r")
        nc.sync.dma_start(out=b1r[:], in_=b1_d[:].to_broadcast([128, C]))
        W2 = cpool.tile([C, NCLS], f32, tag="W2")
        nc.sync.dma_start(out=W2[:], in_=W2_d[:])
        b2r = cpool.tile([128, NCLS], f32, tag="b2r")
        nc.sync.dma_start(out=b2r[:], in_=b2_d[:].to_broadcast([128, NCLS]))
        gidrel = cpool.tile([128, NW], f32, tag="gidrel")
        nc.sync.dma_start(out=gidrel[:], in_=gidrel_d[:])
        gscat = cpool.tile([128, 1], i32, tag="gscat")
        nc.sync.dma_start(out=gscat[:], in_=gscat_d[:])

        adst = [cpool.tile([128, NW * 4], f32, tag=f"adst{i}",
                           name=f"adst{i}") for i in range(2)]
        pool_ps = pp1.tile([128, 65], f32, tag="pool")

        def drain_common(w, ha_ps, tgt):
            ha_t = sp.tile([128, HCA], f32, tag="ha_t")
            nc.vector.tensor_copy(ha_t[:], ha_ps[:, 0:HCA])
            nc.sync.dma_start(out=h_loc[w * WN:(w + 1) * WN, :], in_=ha_t[:])
            nc.vector.tensor_copy(adst[tgt][:, w * 4:(w + 1) * 4],
                                  ha_ps[:, HC + 4:HC + 8])

        def chunk_ags(w, tgt):
            if (w + 1) % CW == 0:
                k = w // CW
                nc.gpsimd.collective_compute(
                    "AllGather", Alu.bypass, replica_groups=RG,
                    ins=[h_loc[k * CHR:(k + 1) * CHR, :]],
                    outs=[h_ext[tgt][k * NC * CHR:(k + 1) * NC * CHR, :]])

        pool_started = [False]

        # -------- layer 0 prologue
        for w in range(NW):
            ha_ps = pp1.tile([128, NHW], f32, tag="ha")
            nc.tensor.matmul(ha_ps[:], xT[:, w * WN:(w + 1) * WN], rhs0[:],
                             start=True, stop=True)
            drain_common(w, ha_ps, 0)
            chunk_ags(w, 0)

        # -------- conv layers
        for l in list(range(3)) * layers_mult:
            cur, nxt = l % 2, (l + 1) % 2
            hf = h_ext[cur]
            for w in range(NW):
                out_ps = pp.tile([128, HCA], f32, tag="outwin")
                ih = sp.tile([128, T], i32, tag="idxh")
                nc.sync.dma_start(out=ih[:], in_=idxs_d[w])
                dc = sp.tile([128, T], f32, tag="drc")
                nc.sync.dma_start(out=dc[:], in_=drc_d[w])
                TW = Tw[w]
                for t in range(TW):
                    g = gpl.tile([128, HCA], f32, tag="g")
                    nc.gpsimd.indirect_dma_start(
                        out=g[:], out_offset=None, in_=hf[:],
                        in_offset=IOff(ap=ih[:, t:t + 1], axis=0))
                    s2 = sp3.tile([128, 128], f32, tag="s2")
                    nc.vector.tensor_scalar(
                        s2[:], iota_row[:], dc[:, t:t + 1], None,
                        op0=Alu.is_equal)
                    s2t_ps = pp.tile([128, 128], f32, tag="ps128")
                    nc.tensor.transpose(s2t_ps[:], s2[:], ident[:])
                    s2t = sp3.tile([128, 128], f32, tag="s2t")
                    nc.vector.tensor_copy(s2t[:], s2t_ps[:])
                    adp = pp.tile([128, 4], f32, tag="adp")
                    nc.tensor.matmul(adp[:], s2t[:],
                                     adst[cur][:, w * 4:(w + 1) * 4],
                                     start=True, stop=True)
                    z = sp3.tile([128, 4], f32, tag="z")
                    nc.vector.tensor_tensor(z[:], g[:, HC:HCA], adp[:],
                                            op=Alu.add)
                    z2 = sp3.tile([128, 4], f32, tag="z2")
                    nc.gpsimd.tensor_scalar_mul(z2[:], z[:], NEG_SLOPE)
                    z3 = sp3.tile([128, 4], f32, tag="z3")
                    nc.vector.tensor_tensor(z3[:], z[:], z2[:], op=Alu.max)
                    ex = sp3.tile([128, 4], f32, tag="ex")
                    nc.scalar.activation(ex[:], z3[:], Act.Exp)
                    msg = sp3.tile([128, HCA], f32, tag="msg")
                    for hh in range(H):
                        nc.scalar.activation(
                            msg[:, hh * C:(hh + 1) * C],
                            g[:, hh * C:(hh + 1) * C], Act.Copy,
                            bias=0.0, scale=ex[:, hh:hh + 1])
                    nc.vector.tensor_copy(msg[:, HC:HCA], ex[:])
                    nc.tensor.matmul(out_ps[:], s2[:], msg[:],
                                     start=(t == 0), stop=(t == TW - 1))
                # ---- drain window w
                den = sp3.tile([128, 4], f32, tag="den")
                nc.vector.tensor_scalar(den[:], out_ps[:, HC:HCA], 1e-16,
                                        None, op0=Alu.add)
                rec = sp3.tile([128, 4], f32, tag="rec")
                nc.vector.reciprocal(rec[:], den[:])
                xw = sp.tile([128, HC], f32, tag="xw")
                nc.vector.tensor_tensor(
                    xw[:].rearrange("p (a b) -> p a b", a=H),
                    out_ps[:, 0:HC].rearrange("p (a b) -> p a b", a=H),
                    rec[:].to_broadcast([128, H, C]), op=Alu.mult)
                xs = sp.tile([128, HC], f32, tag="xs")
                nc.vector.tensor_tensor(xs[:], xw[:], drs[:, l, :],
                                        op=Alu.mult)
                xb = sp.tile([128, HC], f32, tag="xb")
                nc.vector.tensor_tensor(xb[:], xs[:], drb[:, l, :],
                                        op=Alu.add)
                x_t = sp.tile([128, HC], f32, tag="x_t")
                nc.scalar.activation(x_t[:], xb[:], Act.Relu)
                xTa = []
                for cki in range(2):
                    tp = pp.tile([128, 128], f32, tag="ps128")
                    nc.tensor.transpose(
                        tp[:], x_t[:, cki * 128:(cki + 1) * 128], ident[:])
                    xt_s = sp.tile([128, 128], f32, tag=f"xT{cki}",
                                   name=f"xTs{cki}")
                    nc.vector.tensor_copy(xt_s[:], tp[:])
                    xTa.append(xt_s)
                if l < 2:
                    ha_ps = pp1.tile([128, NHW], f32, tag="ha")
                    for cki in range(2):
                        nc.tensor.matmul(ha_ps[:], xTa[cki][:],
                                         rhs12[l][:, cki, :],
                                         start=(cki == 0), stop=(cki == 1))
                    drain_common(w, ha_ps, nxt)
                    chunk_ags(w, nxt)
                else:
                    y_ps = pp1.tile([128, C], f32, tag="ha")
                    for cki in range(2):
                        nc.tensor.matmul(y_ps[:], xTa[cki][:],
                                         rhsF[:, cki, :],
                                         start=(cki == 0), stop=(cki == 1))
                    yb = sp.tile([128, C], f32, tag="yb")
                    nc.vector.tensor_tensor(yb[:], y_ps[:], bfr[:],
                                            op=Alu.add)
                    ya = sp.tile([128, C + 1], f32, tag="ya")
                    nc.scalar.activation(ya[:, 0:C], yb[:], Act.Relu)
                    nc.gpsimd.memset(ya[:, C:C + 1], 1.0)
                    s2g = sp.tile([128, 128], f32, tag="s2g")
                    nc.vector.tensor_scalar(s2g[:], iota_row[:],
                                            gidrel[:, w:w + 1], None,
                                            op0=Alu.is_equal)
                    nc.tensor.matmul(pool_ps[:], s2g[:], ya[:],
                                     start=(w == 0 and not pool_started[0]),
                                     stop=(w == NW - 1), skip_group_check=True)
                    if w == 0:
                        pool_started[0] = True

        # -------- pooling + classifier
        zt = sp.tile([128, 128], f32, tag="zt")
        nc.gpsimd.memset(zt[:], 0.0)
        for i in range(PT // 128):
            nc.sync.dma_start(out=pool_g[i * 128:(i + 1) * 128, :], in_=zt[:])
        psb = sp.tile([128, 128], f32, tag="psb")
        nc.gpsimd.memset(psb[:], 0.0)
        nc.vector.tensor_copy(psb[:, 0:65], pool_ps[:])
        nc.gpsimd.indirect_dma_start(
            out=pool_g[:], out_offset=IOff(ap=gscat[:], axis=0),
            in_=psb[:], in_offset=None, compute_op=Alu.add)
        nc.gpsimd.collective_compute(
            "AllReduce", Alu.add, replica_groups=RG,
            ins=[pool_g[:]], outs=[pool_red[:]])

        for gt in range(NGT):
            rows = min(128, cfg.G - gt * 128)
            pg = sp.tile([128, 65], f32, tag="pg")
            nc.sync.dma_start(out=pg[:],
                              in_=pool_red[gt * 128:gt * 128 + 128, 0:65])
            cntm = sp.tile([128, 1], f32, tag="cntm")
            nc.vector.tensor_scalar_max(cntm[:], pg[:, 64:65], 1.0)
            crec = sp.tile([128, 1], f32, tag="crec")
            nc.vector.reciprocal(crec[:], cntm[:])
            g_t = sp.tile([128, C], f32, tag="g_t")
            nc.vector.tensor_scalar(g_t[:], pg[:, 0:C], crec[:, 0:1], None,
                                    op0=Alu.mult)
            tp = pp.tile([128, 128], f32, tag="ps128")
            nc.tensor.transpose(tp[0:C, :], g_t[:], ident[:])
            gT = sp.tile([C, 128], f32, tag="gT")
            nc.vector.tensor_copy(gT[:], tp[0:C, :])
            z1 = pp1.tile([128, C], f32, tag="ha")
            nc.tensor.matmul(z1[:], gT[:], W1[:], start=True, stop=True)
            r1b = sp.tile([128, C], f32, tag="r1b")
            nc.vector.tensor_tensor(r1b[:], z1[:], b1r[:], op=Alu.add)
            r1 = sp.tile([128, C], f32, tag="r1")
            nc.scalar.activation(r1[:], r1b[:], Act.Relu)
            tp2 = pp.tile([128, 128], f32, tag="ps128")
            nc.tensor.transpose(tp2[0:C, :], r1[:], ident[:])
            r1T = sp.tile([C, 128], f32, tag="r1T")
            nc.vector.tensor_copy(r1T[:], tp2[0:C, :])
            z2p = pp1.tile([128, NCLS], f32, tag="ha")
            nc.tensor.matmul(z2p[:], r1T[:], W2[:], start=True, stop=True)
            ob = sp.tile([128, NCLS], f32, tag="ob")
            nc.vector.tensor_tensor(ob[:], z2p[:], b2r[:], op=Alu.add)
            nc.sync.dma_start(out=out_d[gt * 128:gt * 128 + rows, :],
                              in_=ob[0:rows, :])

    if split_waits:
        _split_multi_waits(nc)
    return nc


# ---------------------------------------------------------------- entry point
def kernel(x, edge_index, batch, params):
    cfg = FULL
    in_maps, T, Tw = _preprocess(cfg, x, edge_index, batch, params)
    nc = _build(cfg, T, Tw)
    from concourse.bass_utils import run_bass_kernel_spmd
    res = run_bass_kernel_spmd(nc, in_maps, list(range(cfg.NC)))
    return np.asarray(res.results[0]["out"], np.float32)
